# revision 1
# baseline (speedup 1.0000x reference)
"""Trainium2 Bass kernel for nn_Cluster (vq_codebook soft-membership).

mu[n, k] = (1/d[n,k]) / sum_j (1/d[n,j]),  d = ||x_n - c_k||^2

Strategy (8 NeuronCores, data-parallel over N):
  - Shard features over N (4096 rows/core); replicate centers.
  - d/2 = x.(-c) + x2/2 + c2/2 via the GEMM identity; the 2x scale cancels
    in the normalization.
  - Main matmuls in fp8 e4m3 with DoubleRow perf mode (0.5 PE cycles/row):
    host-measured mu rel-err from e4m3 inputs is ~1.0e-2, inside tolerance.
    The folded-norm augmentation runs as a separate bf16 rank-4 matmul into
    the same PSUM accumulation group (hi/lo split keeps norms exact).
  - Per 128-row tile: 4 DoubleRow matmuls (K=256 each) + 2 aug matmuls,
    interleaved across the two 512-wide PSUM banks.
  - ACT Reciprocal evacuates PSUM -> inv = 2/d with fused row-sum;
    DVE mu = inv * (1/rowsum), written fp16 and upcast on the host.
  - DMAs batched 2 row-tiles per descriptor; the codebook is split into 8
    chunk DMAs so the first matmul starts as early as possible.
"""

import numpy as np

N, DF, KC = 32768, 512, 1024
N_CORES = 8
P = 128
M_LOC = N // N_CORES            # 4096 rows per core
N_MTILES = M_LOC // P           # 32
DC = DF // P                    # 4 contraction chunks
NBANK = 512                     # fp32 PSUM bank width
NH = KC // NBANK                # 2 output halves
TB = 2                          # row-tiles batched per DMA

_cached_nc = None


def _act_reciprocal(nc, bass, mybir, out, in_, accum_out=None):
    """InstActivation(func=Reciprocal): out = 1/in_, accum_out = row-sum(out).

    Emitted directly (bass.scalar.activation refuses Reciprocal as a policy
    guard); accuracy measured on hardware at ~1e-5 rel for mid-range inputs.
    """
    eng = nc.scalar
    inputs = [eng.lower_ap(in_)]
    for arg in (0.0, 1.0, 0.0):  # bias, scale, alpha
        inputs.append(mybir.ImmediateValue(dtype=mybir.dt.float32, value=arg))
    outputs = [eng.lower_ap(out)]
    if accum_out is not None:
        outputs.append(eng.lower_ap(accum_out))
    return eng.add_instruction(
        mybir.InstActivation(
            name=nc.get_next_instruction_name(),
            func=mybir.ActivationFunctionType.Reciprocal,
            ins=inputs,
            outs=outputs,
        )
    )


def _build():
    global _cached_nc
    if _cached_nc is not None:
        return _cached_nc

    import concourse.bass as bass
    import concourse.mybir as mybir
    import concourse.tile as tile
    from concourse import bacc

    F32 = mybir.dt.float32
    F16 = mybir.dt.float16
    BF16 = mybir.dt.bfloat16
    F8 = mybir.dt.float8e4
    DR = mybir.MatmulPerfMode.DoubleRow

    nc = bacc.Bacc("TRN2", target_bir_lowering=False, debug=False,
                   num_devices=N_CORES)

    # xt[mb, p, c, m] = X[(mb*TB+..)*128 + m, c*128 + p]; each batched DMA
    # line (partition p) is TB contiguous 512 B runs.
    xt = nc.dram_tensor("xt", [N_MTILES // TB, P, TB * DC * P], F8,
                        kind="ExternalInput")
    # ctn[h, c, p, k] = -C[h*512 + k, c*128 + p]; 8 chunked DMAs.
    ctn = nc.dram_tensor("ctn", [NH, DC, P, NBANK], F8, kind="ExternalInput")
    aug_l = nc.dram_tensor("aug_l", [4, M_LOC], BF16, kind="ExternalInput")
    aug_r = nc.dram_tensor("aug_r", [4, KC], BF16, kind="ExternalInput")
    mu = nc.dram_tensor("mu", [M_LOC, KC], F16, kind="ExternalOutput")

    with tile.TileContext(nc) as tc:
        with (
            tc.tile_pool(name="constp", bufs=1) as constp,
            tc.tile_pool(name="xp", bufs=4) as xp,
            tc.tile_pool(name="invp", bufs=3) as invp,
            tc.tile_pool(name="outp", bufs=3) as outp,
            tc.tile_pool(name="smallp", bufs=8) as smallp,
            tc.tile_pool(name="psp", bufs=4, space="PSUM") as psp,
        ):
            # First x batch before the codebook so its DMA queue starts hot.
            x_tiles = [xp.tile([P, TB, DC, P], F8, name="x_t0")]
            nc.sync.dma_start(
                x_tiles[0],
                xt[0].rearrange("p (t c m) -> p t c m", t=TB, c=DC))

            ct_t = constp.tile([P, DC, KC], F8)
            for h in range(NH):
                for c in range(DC):
                    nc.sync.dma_start(
                        ct_t[:, c, h * NBANK:(h + 1) * NBANK], ctn[h, c])
            augl_t = constp.tile([4, M_LOC], BF16)
            nc.sync.dma_start(augl_t, aug_l[:])
            augr_t = constp.tile([4, KC], BF16)
            nc.sync.dma_start(augr_t, aug_r[:])

            for mb in range(N_MTILES // TB):
                if mb + 1 < N_MTILES // TB:
                    nxt = xp.tile([P, TB, DC, P], F8, name=f"x_t{mb+1}")
                    nc.sync.dma_start(
                        nxt,
                        xt[mb + 1].rearrange("p (t c m) -> p t c m",
                                             t=TB, c=DC))
                    x_tiles.append(nxt)
                x_t = x_tiles[mb]
                out_t = outp.tile([P, TB, KC], F16)
                for t in range(TB):
                    mt = mb * TB + t
                    ps = psp.tile([P, KC], F32)
                    # Interleave the two 512-wide halves so the PE never
                    # stalls on a single accumulation group boundary.
                    for cp in range(DC // 2):
                        for nh in range(NH):
                            sl = slice(nh * NBANK, (nh + 1) * NBANK)
                            nc.tensor.matmul(
                                ps[:, sl],
                                lhsT=x_t[:, t, 2 * cp:2 * cp + 2, :],
                                rhs=ct_t[:, 2 * cp:2 * cp + 2, sl],
                                start=(cp == 0),
                                stop=False,
                                perf_mode=DR,
                            )
                    for nh in range(NH):
                        sl = slice(nh * NBANK, (nh + 1) * NBANK)
                        nc.tensor.matmul(
                            ps[:, sl],
                            lhsT=augl_t[:, mt * P:(mt + 1) * P],
                            rhs=augr_t[:, sl],
                            start=False,
                            stop=True,
                        )
                    inv_t = invp.tile([P, KC], F32)
                    s_t = smallp.tile([P, 1], F32)
                    _act_reciprocal(nc, bass, mybir, inv_t, ps, accum_out=s_t)
                    r_t = smallp.tile([P, 1], F32)
                    nc.vector.reciprocal(r_t, s_t)
                    nc.vector.tensor_scalar_mul(out_t[:, t, :], inv_t, r_t)
                # One DMA per TB tiles: mu rows [mb*TB*128, (mb+1)*TB*128).
                nc.sync.dma_start(
                    mu[mb * TB * P:(mb + 1) * TB * P, :].rearrange(
                        "(t m) k -> m t k", t=TB),
                    out_t)

    nc.compile()
    _cached_nc = nc
    return nc


def _prep_in_maps(features, centers):
    import ml_dtypes
    import concourse.mybir as mybir

    f8 = mybir.dt.np(mybir.dt.float8e4)
    bf16 = ml_dtypes.bfloat16

    feats = np.ascontiguousarray(features, dtype=np.float32)
    cents = np.ascontiguousarray(centers, dtype=np.float32)
    assert feats.shape == (N, DF) and cents.shape == (KC, DF)

    # ctn[h, c, p, k] = -C[h*512+k, c*128+p]
    ctn = np.ascontiguousarray(
        (-cents.T.astype(f8)).reshape(DC, P, NH, NBANK).transpose(2, 0, 1, 3))
    # hi/lo double-bf16 split of the folded norms keeps them ~fp32-exact.
    x2h = 0.5 * np.einsum("md,md->m", feats, feats)
    c2h = 0.5 * np.einsum("kd,kd->k", cents, cents)
    x2_hi = x2h.astype(bf16)
    x2_lo = (x2h - x2_hi.astype(np.float32)).astype(bf16)
    c2_hi = c2h.astype(bf16)
    c2_lo = (c2h - c2_hi.astype(np.float32)).astype(bf16)
    ones_k = np.ones(KC, bf16)
    aug_r = np.ascontiguousarray(np.stack([ones_k, ones_k, c2_hi, c2_lo]))

    feats8 = feats.astype(f8)
    ones_m = np.ones(M_LOC, bf16)
    in_maps = []
    for c in range(N_CORES):
        sl = slice(c * M_LOC, (c + 1) * M_LOC)
        # xt[mb, p, t, c, m] = X[(mb*TB+t)*128+m, c*128+p]
        xt = np.ascontiguousarray(
            feats8[sl].reshape(N_MTILES // TB, TB, P, DC, P)
            .transpose(0, 4, 1, 3, 2)
        ).reshape(N_MTILES // TB, P, TB * DC * P)
        aug_l = np.ascontiguousarray(
            np.stack([x2_hi[sl], x2_lo[sl], ones_m, ones_m]))
        in_maps.append({"xt": xt, "ctn": ctn, "aug_l": aug_l, "aug_r": aug_r})
    return in_maps


def _run(inputs, trace=False):
    from concourse.bass_utils import run_bass_kernel_spmd

    nc = _build()
    in_maps = _prep_in_maps(inputs["features"], inputs["centers"])
    res = run_bass_kernel_spmd(
        nc, in_maps, core_ids=list(range(N_CORES)), trace=trace)
    out = np.concatenate([r["mu"] for r in res.results], axis=0)
    return out.astype(np.float32), res


def kernel(features, centers):
    out, _ = _run({"features": features, "centers": centers}, trace=False)
    return out



# revision 2
# speedup vs baseline: 1.3121x; 1.3121x over previous
"""Trainium2 Bass kernel for nn_Cluster (vq_codebook soft-membership).

mu[n, k] = (1/d[n,k]) / sum_j (1/d[n,j]),  d = ||x_n - c_k||^2

Strategy (8 NeuronCores, data-parallel over N):
  - Shard features over N (4096 rows/core); replicate centers.
  - d/2 = x.(-c) + c2/2 + x2/2: the GEMM runs in fp8 e4m3 DoubleRow
    (2 contraction rows/cycle); c2/2 enters PSUM via a rank-1 bf16 matmul
    that opens each accumulation group; x2/2 is folded into the ACT
    Reciprocal as a per-partition bias AP (measured exact to ~1e-5).
  - The PE clock sits at the throttled 1.2 GHz until it sees a ~5us
    contiguous burst of matmuls (HAM un-throttle).  A warm-up burst of
    dummy matmuls runs during the input-DMA phase so the real stream
    starts at 2.4 GHz.
  - ACT Reciprocal evacuates PSUM -> inv = 2/d fp16 with fused row-sum;
    DVE: r = 1/rowsum, mu = inv * r written fp16, upcast on the host.
"""

import numpy as np

N, DF, KC = 32768, 512, 1024
N_CORES = 8
P = 128
M_LOC = N // N_CORES            # 4096 rows per core
N_MTILES = M_LOC // P           # 32
DC = DF // P                    # 4 contraction chunks of 128
NBANK = 512                     # fp32 PSUM bank width
NH = KC // NBANK                # 2 output halves
TB = 2                          # row-tiles batched per DMA
N_WARM = 20                     # warm-up matmuls (~7us at cold clock)

_cached_nc = None


def _act_reciprocal(nc, mybir, out, in_, bias=0.0, accum_out=None):
    """InstActivation(func=Reciprocal): out = 1/(in_ + bias), accum_out =
    row-sum(out).  Emitted directly (bass.scalar.activation refuses
    Reciprocal as a policy guard); bias may be a per-partition [128,1] AP.
    Accuracy measured on hardware at ~1e-5 rel."""
    eng = nc.scalar
    inputs = [eng.lower_ap(in_)]
    for arg in (bias, 1.0, 0.0):  # bias, scale, alpha
        if isinstance(arg, float):
            inputs.append(mybir.ImmediateValue(dtype=mybir.dt.float32, value=arg))
        else:
            inputs.append(eng.lower_ap(arg))
    outputs = [eng.lower_ap(out)]
    if accum_out is not None:
        outputs.append(eng.lower_ap(accum_out))
    return eng.add_instruction(
        mybir.InstActivation(
            name=nc.get_next_instruction_name(),
            func=mybir.ActivationFunctionType.Reciprocal,
            ins=inputs,
            outs=outputs,
        )
    )


def _build():
    global _cached_nc
    if _cached_nc is not None:
        return _cached_nc

    import concourse.mybir as mybir
    import concourse.tile as tile
    from concourse import bacc

    F32 = mybir.dt.float32
    F16 = mybir.dt.float16
    BF16 = mybir.dt.bfloat16
    F8 = mybir.dt.float8e4
    DR = mybir.MatmulPerfMode.DoubleRow

    nc = bacc.Bacc("TRN2", target_bir_lowering=False, debug=False,
                   num_devices=N_CORES)

    # xt[mb, p, t, c, m] = X[(mb*TB+t)*128 + m, c*128 + p] in fp8.
    xt = nc.dram_tensor("xt", [N_MTILES // TB, P, TB * DC * P], F8,
                        kind="ExternalInput")
    # ctn[h, c, p, k] = -C[h*512 + k, c*128 + p] in fp8; 8 chunked DMAs.
    ctn = nc.dram_tensor("ctn", [NH, DC, P, NBANK], F8, kind="ExternalInput")
    # c2r[h, k] = ||c_k||^2 / 2 (bf16 row, rank-1 aug rhs).
    c2r = nc.dram_tensor("c2r", [1, KC], BF16, kind="ExternalInput")
    # x2c[p, mt] = ||x_{mt*128+p}||^2 / 2 (fp32, ACT bias per tile).
    x2c = nc.dram_tensor("x2c", [P, N_MTILES], F32, kind="ExternalInput")
    # warm-up fodder (zeros).
    wz = nc.dram_tensor("wz", [P, NBANK], BF16, kind="ExternalInput")
    mu = nc.dram_tensor("mu", [M_LOC, KC], F16, kind="ExternalOutput")

    with tile.TileContext(nc) as tc:
        with (
            tc.tile_pool(name="constp", bufs=1) as constp,
            tc.tile_pool(name="xp", bufs=4) as xp,
            tc.tile_pool(name="invp", bufs=3) as invp,
            tc.tile_pool(name="outp", bufs=3) as outp,
            tc.tile_pool(name="smallp", bufs=8) as smallp,
            tc.tile_pool(name="psp", bufs=3, space="PSUM") as psp,
            tc.tile_pool(name="warmp", bufs=1, space="PSUM") as warmp,
        ):
            # Warm-up fodder first so its DMA lands before everything else.
            wz_t = constp.tile([P, NBANK], BF16)
            nc.sync.dma_start(wz_t, wz[:])

            # First x batch before the codebook so its DMA queue starts hot.
            x_tiles = [xp.tile([P, TB, DC, P], F8, name="x_t0")]
            nc.sync.dma_start(
                x_tiles[0],
                xt[0].rearrange("p (t c m) -> p t c m", t=TB, c=DC))

            ct_t = constp.tile([P, DC, KC], F8)
            for h in range(NH):
                for c in range(DC):
                    nc.sync.dma_start(
                        ct_t[:, c, h * NBANK:(h + 1) * NBANK], ctn[h, c])
            ones_t = constp.tile([1, P], BF16)
            nc.vector.memset(ones_t, 1.0)
            c2_t = constp.tile([1, KC], BF16)
            nc.sync.dma_start(c2_t, c2r[:])
            x2_t = constp.tile([P, N_MTILES], F32)
            nc.sync.dma_start(x2_t, x2c[:])

            # HAM warm-up: contiguous dummy matmuls into a scratch bank.
            # They depend only on the first (small) DMA, so they run while
            # the codebook/x DMAs stream, and push the PE to 2.4 GHz.
            warm_ps = warmp.tile([P, NBANK], F32)
            for i in range(N_WARM):
                nc.tensor.matmul(warm_ps, lhsT=wz_t[:, :P], rhs=wz_t,
                                 start=True, stop=True, skip_group_check=True)

            for mb in range(N_MTILES // TB):
                if mb + 1 < N_MTILES // TB:
                    nxt = xp.tile([P, TB, DC, P], F8, name=f"x_t{mb+1}")
                    nc.sync.dma_start(
                        nxt,
                        xt[mb + 1].rearrange("p (t c m) -> p t c m",
                                             t=TB, c=DC))
                    x_tiles.append(nxt)
                x_t = x_tiles[mb]
                out_t = outp.tile([P, TB, KC], F16)
                for t in range(TB):
                    mt = mb * TB + t
                    ps = psp.tile([P, KC], F32)
                    # Open both accumulation groups with the rank-1 c2 aug
                    # (shared `ones` stationary => one LDWEIGHTS), then the
                    # two DR passes per half.
                    for nh in range(NH):
                        sl = slice(nh * NBANK, (nh + 1) * NBANK)
                        nc.tensor.matmul(
                            ps[:, sl],
                            lhsT=ones_t,
                            rhs=c2_t[:, sl],
                            start=True,
                            stop=False,
                        )
                    for cp in range(DC // 2):
                        for nh in range(NH):
                            sl = slice(nh * NBANK, (nh + 1) * NBANK)
                            nc.tensor.matmul(
                                ps[:, sl],
                                lhsT=x_t[:, t, 2 * cp:2 * cp + 2, :],
                                rhs=ct_t[:, 2 * cp:2 * cp + 2, sl],
                                start=False,
                                stop=(cp == DC // 2 - 1),
                                perf_mode=DR,
                            )
                    inv_t = invp.tile([P, KC], F16)
                    s_t = smallp.tile([P, 1], F32)
                    _act_reciprocal(nc, mybir, inv_t, ps,
                                    bias=x2_t[:, mt:mt + 1], accum_out=s_t)
                    r_t = smallp.tile([P, 1], F32)
                    nc.vector.reciprocal(r_t, s_t)
                    nc.vector.tensor_scalar_mul(out_t[:, t, :], inv_t, r_t)
                # One DMA per TB tiles: mu rows [mb*TB*128, (mb+1)*TB*128).
                nc.sync.dma_start(
                    mu[mb * TB * P:(mb + 1) * TB * P, :].rearrange(
                        "(t m) k -> m t k", t=TB),
                    out_t)

    nc.compile()
    _cached_nc = nc
    return nc


def _prep_in_maps(features, centers):
    import concourse.mybir as mybir
    import ml_dtypes

    f8 = mybir.dt.np(mybir.dt.float8e4)
    bf16 = ml_dtypes.bfloat16

    feats = np.ascontiguousarray(features, dtype=np.float32)
    cents = np.ascontiguousarray(centers, dtype=np.float32)
    assert feats.shape == (N, DF) and cents.shape == (KC, DF)

    # ctn[h, c, p, k] = -C[h*512+k, c*128+p]
    ctn = np.ascontiguousarray(
        (-cents.T.astype(f8)).reshape(DC, P, NH, NBANK).transpose(2, 0, 1, 3))
    x2h = 0.5 * np.einsum("md,md->m", feats, feats)
    c2h = 0.5 * np.einsum("kd,kd->k", cents, cents)
    c2r = np.ascontiguousarray(c2h.astype(bf16)).reshape(1, KC)
    wzv = np.zeros((P, NBANK), bf16)

    feats8 = feats.astype(f8)
    in_maps = []
    for c in range(N_CORES):
        sl = slice(c * M_LOC, (c + 1) * M_LOC)
        # xt[mb, p, t, c, m] = X[(mb*TB+t)*128+m, c*128+p]
        xtc = np.ascontiguousarray(
            feats8[sl].reshape(N_MTILES // TB, TB, P, DC, P)
            .transpose(0, 4, 1, 3, 2)
        ).reshape(N_MTILES // TB, P, TB * DC * P)
        # x2c[p, mt] = x2h[sl][mt*128 + p]
        x2cc = np.ascontiguousarray(
            x2h[sl].astype(np.float32).reshape(N_MTILES, P).T)
        in_maps.append({"xt": xtc, "ctn": ctn, "c2r": c2r, "x2c": x2cc,
                        "wz": wzv})
    return in_maps


def _run(inputs, trace=False):
    from concourse.bass_utils import run_bass_kernel_spmd

    nc = _build()
    in_maps = _prep_in_maps(inputs["features"], inputs["centers"])
    res = run_bass_kernel_spmd(
        nc, in_maps, core_ids=list(range(N_CORES)), trace=trace)
    out = np.concatenate([r["mu"] for r in res.results], axis=0)
    return out.astype(np.float32), res


def kernel(features, centers):
    out, _ = _run({"features": features, "centers": centers}, trace=False)
    return out


# revision 3
# speedup vs baseline: 1.5705x; 1.1969x over previous
"""Trainium2 Bass kernel for nn_Cluster (vq_codebook soft-membership).

mu[n, k] = (1/d[n,k]) / sum_j (1/d[n,j]),  d = ||x_n - c_k||^2

Strategy (8 NeuronCores, data-parallel over N):
  - Shard features over N (4096 rows/core); replicate centers.
  - d/2 = x.(-c) + x2/2 + c2/2 in fp8 e4m3 DoubleRow matmuls (2 contraction
    rows/cycle).  The PE clock sits throttled at 1.2 GHz until it sees a
    ~5us contiguous burst of matmuls (HAM un-throttle), so a warm-up burst
    of dummy matmuls on memset data runs during the input-DMA phase and the
    real stream starts at ~2x the clock.
  - The norm terms (x2/2 + c2/2) enter per tile either
      (a) as a rank-4 bf16 hi/lo aug matmul opening the PSUM accumulation
          group (PE path), or
      (b) as a fused DVE scalar_tensor_tensor (d = (psum + x2) + c2bcast)
          evacuating PSUM to SBUF fp16 (DVE path),
    split ~44/56 across tiles so the PE and DVE pipelines stay balanced.
  - ACT Reciprocal gives inv = 2/d fp16 with fused row-sum; DVE computes
    r = 1/rowsum and mu = inv * r in fp16, upcast on the host.
"""

import numpy as np

N, DF, KC = 32768, 512, 1024
N_CORES = 8
P = 128
M_LOC = N // N_CORES            # 4096 rows per core
N_MTILES = M_LOC // P           # 32
DC = DF // P                    # 4 contraction chunks of 128
NBANK = 512                     # fp32 PSUM bank width
NH = KC // NBANK                # 2 output halves
TB = 2                          # row-tiles batched per DMA
N_WARM = 20                     # warm-up matmuls (~7us at cold clock)
# Tiles whose norm-add runs as a PE aug matmul (rest go via DVE STT).
PE_AUG_TILES = frozenset(
    mt for mt in range(N_MTILES) if mt % 16 in (0, 2, 5, 7, 9, 11, 14))

_cached_nc = None


def _act_reciprocal(nc, mybir, out, in_, accum_out=None):
    """InstActivation(func=Reciprocal): out = 1/in_, accum_out = row-sum.
    Emitted directly (bass.scalar.activation refuses Reciprocal as a policy
    guard); accuracy measured on hardware at ~1e-5 rel."""
    eng = nc.scalar
    inputs = [eng.lower_ap(in_)]
    for arg in (0.0, 1.0, 0.0):  # bias, scale, alpha
        inputs.append(mybir.ImmediateValue(dtype=mybir.dt.float32, value=arg))
    outputs = [eng.lower_ap(out)]
    if accum_out is not None:
        outputs.append(eng.lower_ap(accum_out))
    return eng.add_instruction(
        mybir.InstActivation(
            name=nc.get_next_instruction_name(),
            func=mybir.ActivationFunctionType.Reciprocal,
            ins=inputs,
            outs=outputs,
        )
    )


def _build():
    global _cached_nc
    if _cached_nc is not None:
        return _cached_nc

    import concourse.mybir as mybir
    import concourse.tile as tile
    from concourse import bacc

    F32 = mybir.dt.float32
    F16 = mybir.dt.float16
    BF16 = mybir.dt.bfloat16
    F8 = mybir.dt.float8e4
    DR = mybir.MatmulPerfMode.DoubleRow
    ADD = mybir.AluOpType.add

    nc = bacc.Bacc("TRN2", target_bir_lowering=False, debug=False,
                   num_devices=N_CORES)

    # xt[mb, p, t, c, m] = X[(mb*TB+t)*128 + m, c*128 + p] in fp8.
    xt = nc.dram_tensor("xt", [N_MTILES // TB, P, TB * DC * P], F8,
                        kind="ExternalInput")
    # ctn[c, h, p, k] = -C[h*512 + k, c*128 + p] in fp8; 8 chunked DMAs.
    ctn = nc.dram_tensor("ctn", [DC, NH, P, NBANK], F8, kind="ExternalInput")
    # Rank-4 hi/lo aug: aug_l rows = [x2h, x2l, 1, 1], aug_r = [1, 1, c2h, c2l].
    aug_l = nc.dram_tensor("aug_l", [4, M_LOC], BF16, kind="ExternalInput")
    aug_r = nc.dram_tensor("aug_r", [4, KC], BF16, kind="ExternalInput")
    # x2c[p, mt] = ||x_{mt*128+p}||^2 / 2 (fp32, STT scalar per tile).
    x2c = nc.dram_tensor("x2c", [P, N_MTILES], F32, kind="ExternalInput")
    # c2b[p, k] = ||c_k||^2 / 2 broadcast to all partitions (fp32).
    c2b = nc.dram_tensor("c2b", [P, KC], F32, kind="ExternalInput")
    mu = nc.dram_tensor("mu", [M_LOC, KC], F16, kind="ExternalOutput")

    with tile.TileContext(nc) as tc:
        with (
            tc.tile_pool(name="constp", bufs=1) as constp,
            tc.tile_pool(name="xp", bufs=4) as xp,
            tc.tile_pool(name="invp", bufs=3) as invp,
            tc.tile_pool(name="dp", bufs=3) as dp,
            tc.tile_pool(name="outp", bufs=3) as outp,
            tc.tile_pool(name="smallp", bufs=8) as smallp,
            tc.tile_pool(name="psp", bufs=4, space="PSUM") as psp,
        ):
            # Warm-up fodder built on-chip (no DMA dependency) so the PE
            # burst starts as soon as the runtime preamble finishes.
            wz_t = constp.tile([P, NBANK], BF16)
            nc.vector.memset(wz_t, 0.0)
            warm_ps = psp.tile([P, KC], F32, name="ps")
            for i in range(N_WARM):
                nc.tensor.matmul(warm_ps[:, :NBANK], lhsT=wz_t[:, :P],
                                 rhs=wz_t, start=True, stop=True,
                                 skip_group_check=True)

            # First x batch before the codebook so its DMA queue starts hot.
            x_tiles = [xp.tile([P, TB, DC, P], F8, name="x_t0")]
            nc.sync.dma_start(
                x_tiles[0],
                xt[0].rearrange("p (t c m) -> p t c m", t=TB, c=DC))

            ct_t = constp.tile([P, DC, KC], F8)
            for c in range(DC):
                for h in range(NH):
                    nc.sync.dma_start(
                        ct_t[:, c, h * NBANK:(h + 1) * NBANK], ctn[c, h])
            augl_t = constp.tile([4, M_LOC], BF16)
            nc.sync.dma_start(augl_t, aug_l[:])
            augr_t = constp.tile([4, KC], BF16)
            nc.sync.dma_start(augr_t, aug_r[:])
            x2_t = constp.tile([P, N_MTILES], F32)
            nc.sync.dma_start(x2_t, x2c[:])
            c2b_t = constp.tile([P, KC], F32)
            nc.sync.dma_start(c2b_t, c2b[:])

            for mb in range(N_MTILES // TB):
                if mb + 1 < N_MTILES // TB:
                    nxt = xp.tile([P, TB, DC, P], F8, name=f"x_t{mb+1}")
                    nc.sync.dma_start(
                        nxt,
                        xt[mb + 1].rearrange("p (t c m) -> p t c m",
                                             t=TB, c=DC))
                    x_tiles.append(nxt)
                x_t = x_tiles[mb]
                out_t = outp.tile([P, TB, KC], F16)
                for t in range(TB):
                    mt = mb * TB + t
                    on_pe = mt in PE_AUG_TILES
                    ps = psp.tile([P, KC], F32, name="ps")
                    if on_pe:
                        # Open both groups with the rank-4 hi/lo aug.
                        for nh in range(NH):
                            sl = slice(nh * NBANK, (nh + 1) * NBANK)
                            nc.tensor.matmul(
                                ps[:, sl],
                                lhsT=augl_t[:, mt * P:(mt + 1) * P],
                                rhs=augr_t[:, sl],
                                start=True,
                                stop=False,
                            )
                    for cp in range(DC // 2):
                        for nh in range(NH):
                            sl = slice(nh * NBANK, (nh + 1) * NBANK)
                            nc.tensor.matmul(
                                ps[:, sl],
                                lhsT=x_t[:, t, 2 * cp:2 * cp + 2, :],
                                rhs=ct_t[:, 2 * cp:2 * cp + 2, sl],
                                start=(not on_pe and cp == 0),
                                stop=(cp == DC // 2 - 1),
                                perf_mode=DR,
                            )
                    inv_t = invp.tile([P, KC], F16)
                    s_t = smallp.tile([P, 1], F32)
                    if on_pe:
                        _act_reciprocal(nc, mybir, inv_t, ps, accum_out=s_t)
                    else:
                        # DVE evacuates PSUM with the fused norm-add, then
                        # ACT runs the reciprocal from SBUF.
                        d_t = dp.tile([P, KC], F16)
                        nc.vector.scalar_tensor_tensor(
                            d_t, ps, x2_t[:, mt:mt + 1], c2b_t,
                            op0=ADD, op1=ADD)
                        _act_reciprocal(nc, mybir, inv_t, d_t, accum_out=s_t)
                    r_t = smallp.tile([P, 1], F32)
                    nc.vector.reciprocal(r_t, s_t)
                    nc.vector.tensor_scalar_mul(out_t[:, t, :], inv_t, r_t)
                # One DMA per TB tiles: mu rows [mb*TB*128, (mb+1)*TB*128).
                nc.sync.dma_start(
                    mu[mb * TB * P:(mb + 1) * TB * P, :].rearrange(
                        "(t m) k -> m t k", t=TB),
                    out_t)

    nc.compile()
    _cached_nc = nc
    return nc


def _prep_in_maps(features, centers):
    import concourse.mybir as mybir
    import ml_dtypes

    f8 = mybir.dt.np(mybir.dt.float8e4)
    bf16 = ml_dtypes.bfloat16

    feats = np.ascontiguousarray(features, dtype=np.float32)
    cents = np.ascontiguousarray(centers, dtype=np.float32)
    assert feats.shape == (N, DF) and cents.shape == (KC, DF)

    # ctn[c, h, p, k] = -C[h*512+k, c*128+p]
    ctn = np.ascontiguousarray(
        (-cents.T.astype(f8)).reshape(DC, P, NH, NBANK).transpose(0, 2, 1, 3))
    x2h = 0.5 * np.einsum("md,md->m", feats, feats)
    c2h = 0.5 * np.einsum("kd,kd->k", cents, cents)
    # hi/lo double-bf16 split keeps the aug-matmul norms ~fp32-exact.
    c2_hi = c2h.astype(bf16)
    c2_lo = (c2h - c2_hi.astype(np.float32)).astype(bf16)
    ones_k = np.ones(KC, bf16)
    aug_r = np.ascontiguousarray(np.stack([ones_k, ones_k, c2_hi, c2_lo]))
    c2b = np.ascontiguousarray(
        np.broadcast_to(c2h.astype(np.float32), (P, KC)))

    feats8 = feats.astype(f8)
    ones_m = np.ones(M_LOC, bf16)
    in_maps = []
    for c in range(N_CORES):
        sl = slice(c * M_LOC, (c + 1) * M_LOC)
        # xt[mb, p, t, c, m] = X[(mb*TB+t)*128+m, c*128+p]
        xtc = np.ascontiguousarray(
            feats8[sl].reshape(N_MTILES // TB, TB, P, DC, P)
            .transpose(0, 4, 1, 3, 2)
        ).reshape(N_MTILES // TB, P, TB * DC * P)
        x2_hi = x2h[sl].astype(bf16)
        x2_lo = (x2h[sl] - x2_hi.astype(np.float32)).astype(bf16)
        aug_l = np.ascontiguousarray(np.stack([x2_hi, x2_lo, ones_m, ones_m]))
        x2cc = np.ascontiguousarray(
            x2h[sl].astype(np.float32).reshape(N_MTILES, P).T)
        in_maps.append({"xt": xtc, "ctn": ctn, "aug_l": aug_l, "aug_r": aug_r,
                        "x2c": x2cc, "c2b": c2b})
    return in_maps


def _run(inputs, trace=False):
    from concourse.bass_utils import run_bass_kernel_spmd

    nc = _build()
    in_maps = _prep_in_maps(inputs["features"], inputs["centers"])
    res = run_bass_kernel_spmd(
        nc, in_maps, core_ids=list(range(N_CORES)), trace=trace)
    out = np.concatenate([r["mu"] for r in res.results], axis=0)
    return out.astype(np.float32), res


def kernel(features, centers):
    out, _ = _run({"features": features, "centers": centers}, trace=False)
    return out


# revision 13
# speedup vs baseline: 1.7262x; 1.0992x over previous
"""Trainium2 Bass kernel for nn_Cluster (vq_codebook soft-membership).

mu[n, k] = (1/d[n,k]) / sum_j (1/d[n,j]),  d = ||x_n - c_k||^2

Strategy (8 NeuronCores, data-parallel over N):
  - Shard features over N (4096 rows/core); replicate centers.
  - d/2 = x.(-c) + x2/2 + c2/2 in fp8 e4m3 DoubleRow matmuls (2 contraction
    rows/cycle).  The PE clock sits throttled at 1.2 GHz until it sees a
    ~5us contiguous burst of matmuls (HAM un-throttle), so a warm-up burst
    of dummy matmuls on memset data runs during the input-DMA phase and the
    real stream starts at ~2x the clock.
  - The norm terms (x2/2 + c2/2) enter per tile either
      (a) as a rank-4 bf16 hi/lo aug matmul opening the PSUM accumulation
          group (PE path), or
      (b) as a fused DVE scalar_tensor_tensor (d = (psum + x2) + c2bcast)
          evacuating PSUM to SBUF fp16 (DVE path),
    split ~44/56 across tiles so the PE and DVE pipelines stay balanced.
  - ACT Reciprocal gives inv = 2/d fp16 with fused row-sum; DVE computes
    r = 1/rowsum and mu = inv * r in fp16, upcast on the host.
"""

import numpy as np

N, DF, KC = 32768, 512, 1024
N_CORES = 8
P = 128
M_LOC = N // N_CORES            # 4096 rows per core
N_MTILES = M_LOC // P           # 32
DC = DF // P                    # 4 contraction chunks of 128
NBANK = 512                     # fp32 PSUM bank width
NH = KC // NBANK                # 2 output halves
TB = 2                          # row-tiles batched per DMA
N_WARM = 20                     # warm-up matmuls (~7us at cold clock)
XB = 4                          # row-tiles per input DMA batch (2KB lines)
# Tiles whose norm-add runs as a PE aug matmul (rest go via DVE STT).
PE_AUG_TILES = frozenset(
    mt for mt in range(N_MTILES) if mt % 16 in (0, 2, 5, 7, 9, 11, 14))
# DVE-path tiles whose inv row-sum runs on the (otherwise idle) GPSIMD.
# (walrus's checkTensorScalarPtr rejects accum_out on the Pool engine, so
# this stays empty; kept for documentation of the attempt.)
GP_SUM_TILES = frozenset()

_cached_nc = None


def _act_reciprocal(nc, mybir, out, in_, accum_out=None):
    """InstActivation(func=Reciprocal): out = 1/in_, accum_out = row-sum.
    Emitted directly (bass.scalar.activation refuses Reciprocal as a policy
    guard); accuracy measured on hardware at ~1e-5 rel."""
    eng = nc.scalar
    inputs = [eng.lower_ap(in_)]
    for arg in (0.0, 1.0, 0.0):  # bias, scale, alpha
        inputs.append(mybir.ImmediateValue(dtype=mybir.dt.float32, value=arg))
    outputs = [eng.lower_ap(out)]
    if accum_out is not None:
        outputs.append(eng.lower_ap(accum_out))
    return eng.add_instruction(
        mybir.InstActivation(
            name=nc.get_next_instruction_name(),
            func=mybir.ActivationFunctionType.Reciprocal,
            ins=inputs,
            outs=outputs,
        )
    )


def _build():
    global _cached_nc
    if _cached_nc is not None:
        return _cached_nc

    import concourse.mybir as mybir
    import concourse.tile as tile
    from concourse import bacc

    F32 = mybir.dt.float32
    F16 = mybir.dt.float16
    BF16 = mybir.dt.bfloat16
    F8 = mybir.dt.float8e4
    DR = mybir.MatmulPerfMode.DoubleRow
    ADD = mybir.AluOpType.add

    nc = bacc.Bacc("TRN2", target_bir_lowering=False, debug=False,
                   num_devices=N_CORES)

    # xt[mb, p, t, c, m] = X[(mb*XB+t)*128 + m, c*128 + p] in fp8.
    xt = nc.dram_tensor("xt", [N_MTILES // XB, P, XB * DC * P], F8,
                        kind="ExternalInput")
    # ctn[p, c, k] = -C[k, c*128 + p] in fp8; one 4KB-per-line DMA.
    ctn = nc.dram_tensor("ctn", [P, DC * KC], F8, kind="ExternalInput")
    # Rank-4 hi/lo aug: aug_l rows = [x2h, x2l, 1, 1], aug_r = [1, 1, c2h, c2l].
    aug_l = nc.dram_tensor("aug_l", [4, M_LOC], BF16, kind="ExternalInput")
    aug_r = nc.dram_tensor("aug_r", [4, KC], BF16, kind="ExternalInput")
    # x2c[p, mt] = ||x_{mt*128+p}||^2 / 2 (fp32, STT scalar per tile).
    x2c = nc.dram_tensor("x2c", [P, N_MTILES], F32, kind="ExternalInput")
    # c2b[p, k] = ||c_k||^2 / 2 broadcast to all partitions (fp32).
    c2b = nc.dram_tensor("c2b", [P, KC], F32, kind="ExternalInput")
    mu = nc.dram_tensor("mu", [M_LOC, KC], F16, kind="ExternalOutput")

    with tile.TileContext(nc) as tc:
        with (
            tc.tile_pool(name="constp", bufs=1) as constp,
            tc.tile_pool(name="xp", bufs=4) as xp,
            tc.tile_pool(name="invp", bufs=3) as invp,
            tc.tile_pool(name="dp", bufs=3) as dp,
            tc.tile_pool(name="outp", bufs=3) as outp,
            tc.tile_pool(name="smallp", bufs=8) as smallp,
            tc.tile_pool(name="gscp", bufs=2) as gscp,
            tc.tile_pool(name="psp", bufs=4, space="PSUM") as psp,
        ):
            # Warm-up fodder built on-chip (no DMA dependency) so the PE
            # burst starts as soon as the runtime preamble finishes.
            wz_t = constp.tile([P, NBANK], BF16)
            nc.vector.memset(wz_t, 0.0)
            warm_ps = psp.tile([P, KC], F32, name="ps")
            for i in range(N_WARM):
                nc.tensor.matmul(warm_ps[:, :NBANK], lhsT=wz_t[:, :P],
                                 rhs=wz_t, start=True, stop=True,
                                 skip_group_check=True)

            # First x batch before the codebook so its DMA queue starts hot.
            x_tiles = [xp.tile([P, XB, DC, P], F8, name="x_t0")]
            nc.sync.dma_start(
                x_tiles[0],
                xt[0].rearrange("p (t c m) -> p t c m", t=XB, c=DC))

            ct_t = constp.tile([P, DC, KC], F8)
            nc.sync.dma_start(
                ct_t, ctn[:].rearrange("p (c k) -> p c k", c=DC))
            augl_t = constp.tile([4, M_LOC], BF16)
            nc.sync.dma_start(augl_t, aug_l[:])
            augr_t = constp.tile([4, KC], BF16)
            nc.sync.dma_start(augr_t, aug_r[:])
            x2_t = constp.tile([P, N_MTILES], F32)
            nc.sync.dma_start(x2_t, x2c[:])
            c2b_t = constp.tile([P, KC], F32)
            nc.sync.dma_start(c2b_t, c2b[:])

            for mb in range(N_MTILES // TB):
                xb = (mb * TB) // XB
                if mb * TB % XB == 0 and xb + 1 < N_MTILES // XB:
                    nxt = xp.tile([P, XB, DC, P], F8, name=f"x_t{xb+1}")
                    nc.sync.dma_start(
                        nxt,
                        xt[xb + 1].rearrange("p (t c m) -> p t c m",
                                             t=XB, c=DC))
                    x_tiles.append(nxt)
                x_t = x_tiles[xb]
                out_t = outp.tile([P, TB, KC], F16)
                for t in range(TB):
                    mt = mb * TB + t
                    xs = mt % XB
                    on_pe = mt in PE_AUG_TILES
                    ps = psp.tile([P, KC], F32, name="ps")
                    if on_pe:
                        # Open both groups with the rank-4 hi/lo aug.
                        for nh in range(NH):
                            sl = slice(nh * NBANK, (nh + 1) * NBANK)
                            nc.tensor.matmul(
                                ps[:, sl],
                                lhsT=augl_t[:, mt * P:(mt + 1) * P],
                                rhs=augr_t[:, sl],
                                start=True,
                                stop=False,
                            )
                    for cp in range(DC // 2):
                        for nh in range(NH):
                            sl = slice(nh * NBANK, (nh + 1) * NBANK)
                            nc.tensor.matmul(
                                ps[:, sl],
                                lhsT=x_t[:, xs, 2 * cp:2 * cp + 2, :],
                                rhs=ct_t[:, 2 * cp:2 * cp + 2, sl],
                                start=(not on_pe and cp == 0),
                                stop=(cp == DC // 2 - 1),
                                perf_mode=DR,
                            )
                    inv_t = invp.tile([P, KC], F16)
                    s_t = smallp.tile([P, 1], F32)
                    on_gp = (not on_pe) and mt in GP_SUM_TILES
                    if on_pe:
                        _act_reciprocal(nc, mybir, inv_t, ps, accum_out=s_t)
                    else:
                        # DVE evacuates PSUM with the fused norm-add, then
                        # ACT runs the reciprocal from SBUF.
                        d_t = dp.tile([P, KC], F16)
                        nc.vector.scalar_tensor_tensor(
                            d_t, ps, x2_t[:, mt:mt + 1], c2b_t,
                            op0=ADD, op1=ADD)
                        _act_reciprocal(nc, mybir, inv_t, d_t,
                                        accum_out=None if on_gp else s_t)
                    if on_gp:
                        # Row-sum on the otherwise idle GPSIMD engine to
                        # spare ScalarE the ACT accumulator-read (free-axis
                        # reduce isn't a GPSIMD op, so ride tensor_scalar's
                        # accum_out; the main output lands in a scratch).
                        gsc_t = gscp.tile([P, KC], F16)
                        nc.gpsimd.tensor_scalar(
                            gsc_t, inv_t, 0.0, None, op0=ADD,
                            accum_out=s_t)
                    r_t = smallp.tile([P, 1], F32)
                    nc.vector.reciprocal(r_t, s_t)
                    nc.vector.tensor_scalar_mul(out_t[:, t, :], inv_t, r_t)
                # One DMA per TB tiles: mu rows [mb*TB*128, (mb+1)*TB*128).
                nc.sync.dma_start(
                    mu[mb * TB * P:(mb + 1) * TB * P, :].rearrange(
                        "(t m) k -> m t k", t=TB),
                    out_t)

    nc.compile()
    _cached_nc = nc
    return nc


def _prep_in_maps(features, centers):
    import concourse.mybir as mybir
    import ml_dtypes

    f8 = mybir.dt.np(mybir.dt.float8e4)
    bf16 = ml_dtypes.bfloat16

    feats = np.ascontiguousarray(features, dtype=np.float32)
    cents = np.ascontiguousarray(centers, dtype=np.float32)
    assert feats.shape == (N, DF) and cents.shape == (KC, DF)

    # ctn[p, c, k] = -C[k, c*128+p]
    ctn = np.ascontiguousarray(
        (-cents.T.astype(f8)).reshape(DC, P, KC).transpose(1, 0, 2)
    ).reshape(P, DC * KC)
    x2h = 0.5 * np.einsum("md,md->m", feats, feats)
    c2h = 0.5 * np.einsum("kd,kd->k", cents, cents)
    # hi/lo double-bf16 split keeps the aug-matmul norms ~fp32-exact.
    c2_hi = c2h.astype(bf16)
    c2_lo = (c2h - c2_hi.astype(np.float32)).astype(bf16)
    ones_k = np.ones(KC, bf16)
    aug_r = np.ascontiguousarray(np.stack([ones_k, ones_k, c2_hi, c2_lo]))
    c2b = np.ascontiguousarray(
        np.broadcast_to(c2h.astype(np.float32), (P, KC)))

    feats8 = feats.astype(f8)
    ones_m = np.ones(M_LOC, bf16)
    in_maps = []
    for c in range(N_CORES):
        sl = slice(c * M_LOC, (c + 1) * M_LOC)
        # xt[mb, p, t, c, m] = X[(mb*XB+t)*128+m, c*128+p]
        xtc = np.ascontiguousarray(
            feats8[sl].reshape(N_MTILES // XB, XB, P, DC, P)
            .transpose(0, 4, 1, 3, 2)
        ).reshape(N_MTILES // XB, P, XB * DC * P)
        x2_hi = x2h[sl].astype(bf16)
        x2_lo = (x2h[sl] - x2_hi.astype(np.float32)).astype(bf16)
        aug_l = np.ascontiguousarray(np.stack([x2_hi, x2_lo, ones_m, ones_m]))
        x2cc = np.ascontiguousarray(
            x2h[sl].astype(np.float32).reshape(N_MTILES, P).T)
        in_maps.append({"xt": xtc, "ctn": ctn, "aug_l": aug_l, "aug_r": aug_r,
                        "x2c": x2cc, "c2b": c2b})
    return in_maps


def _run(inputs, trace=False):
    from concourse.bass_utils import run_bass_kernel_spmd

    nc = _build()
    in_maps = _prep_in_maps(inputs["features"], inputs["centers"])
    res = run_bass_kernel_spmd(
        nc, in_maps, core_ids=list(range(N_CORES)), trace=trace)
    out = np.concatenate([r["mu"] for r in res.results], axis=0)
    return out.astype(np.float32), res


def kernel(features, centers):
    out, _ = _run({"features": features, "centers": centers}, trace=False)
    return out
